# revision 27
# baseline (speedup 1.0000x reference)
"""MoE experts kernel for Trainium2 (Bass/Tile), expert-parallel across 8 NeuronCores.

Problem: nn_CompressedMoeExperts — T=2048 tokens, D=1024, FF=1536, E=8 experts,
top-k=2.  out[t] = sum_e combine[e,t] * (silu(h[t] @ Wg[e].T) * (h[t] @ Wu[e].T)) @ Wd[e].T

Sharding: expert-parallel with FF-split load balancing.  Each expert's MLP is
split into two independent shards along the FF dimension (rows of Wg/Wu,
columns of Wd — their partial down-projection outputs simply add).  The 16
shards are sorted by routed-token count and dealt out so every core gets one
"big" and one "small" shard; per-core compute is then proportional to
C0+C1 (max big + max small capacity) instead of 2*C_max, which removes the
hot-expert straggler penalty under skewed routing.  Token dispatch (gather by
top_k_index) and the weighted combine scatter-add happen on the host as part
of sharding/unsharding; the combine weight itself is applied on-device.

Matmul operands are fp16 (halves HBM traffic vs fp32, 1 cycle/row on the PE,
fast weight loads — unlike float32r which forces a ~190ns LDWEIGHTS reload per
matmul), accumulating in fp32 PSUM.  Values are far inside fp16 range and the
10-bit mantissa keeps L2 relative error ~5e-4.  All DMA feeds are pre-laid-out
on the host into exact SBUF tile layouts so every DMA is contiguous, split into
~0.5MB pieces to spread across DMA queues.  A short run of dummy matmuls warms
the PE clock (HAM gate: 1.2 GHz until ~3.4us of sustained activity) while the
first DMAs stage.
"""

import os
import sys

sys.path.insert(0, "/opt/trn_rl_repo")

import numpy as np

import concourse.bass as bass
import concourse.mybir as mybir
import concourse.tile as tile
from concourse import bacc
from concourse.bass_utils import run_bass_kernel_spmd

# Fixed problem shape
T, D, FF, E, TOPK = 2048, 1024, 1536, 8, 2
P = 128
DSUB = D // P     # 8   k-subtiles over the D contraction
FBLK = FF // P    # 12  128-row blocks over the full FF dimension
NSPLIT = 4        # FF-shards per expert (= shard slots per core)
FBH = FBLK // NSPLIT   # 128-row FF blocks per shard
FH = FF // NSPLIT      # FF columns per shard
NDN = 512         # free-dim tile for the down projection
NDT = D // NDN    # 2

F32 = mybir.dt.float32
F16 = mybir.dt.float16

_program_cache: dict[tuple, "bass.Bass"] = {}
last_results = None  # BassKernelResults of the most recent run (for profiling)


def _chunks(C: int) -> list[int]:
    """Split C (multiple of 128) into matmul moving-dim chunks of <=512
    (PSUM bank limit for fp32 accumulation)."""
    nb = C // P
    n = -(-nb * P // 512)  # ceil(C/512)
    base, rem = divmod(nb, n)
    return [(base + (1 if i < rem else 0)) * P for i in range(n)]


def _build_program(Cs: tuple) -> "bass.Bass":
    nc = bacc.Bacc(None, target_bir_lowering=False)

    xt_d = [
        nc.dram_tensor(f"xt{s}", [P, DSUB, Cs[s]], F16, kind="ExternalInput")
        for s in range(NSPLIT)
    ]
    wg_d = nc.dram_tensor("wg", [FBLK, P, DSUB, P], F16, kind="ExternalInput")
    wu_d = nc.dram_tensor("wu", [FBLK, P, DSUB, P], F16, kind="ExternalInput")
    wd_d = nc.dram_tensor("wd", [FBLK, P, NDT, NDN], F16, kind="ExternalInput")
    wt_d = [
        nc.dram_tensor(f"wt{s}", [P, Cs[s] // P], F32, kind="ExternalInput")
        for s in range(NSPLIT)
    ]
    y_d = [
        nc.dram_tensor(f"y{s}", [Cs[s] // P, P, D], F32, kind="ExternalOutput")
        for s in range(NSPLIT)
    ]

    with tile.TileContext(nc) as tc:
        with (
            tc.tile_pool(name="const", bufs=1) as const_pool,
            tc.tile_pool(name="wpool", bufs=3) as wpool,
            tc.tile_pool(name="actp", bufs=1) as act_pool,
            tc.tile_pool(name="sgp", bufs=3) as sg_pool,
            tc.tile_pool(name="yp", bufs=3) as y_pool,
            tc.tile_pool(name="psum", bufs=2, space="PSUM") as psum_pool,
            tc.tile_pool(name="psum_y", bufs=3, space="PSUM") as psum_y_pool,
            tc.tile_pool(name="psum_w", bufs=1, space="PSUM") as psum_w_pool,
        ):
            # HAM pre-warm: dummy matmuls (only dep: the memset) keep the PE
            # busy while the first DMAs stage, so real matmuls start at 2.4GHz.
            warm_in = const_pool.tile([P, NDN], F16)
            nc.vector.memset(warm_in[:], 0.0)
            warm_ps = psum_w_pool.tile([P, NDN], F32)
            for _ in range(10):
                nc.tensor.matmul(warm_ps[:], warm_in[:, :P], warm_in[:])

            # First weight block, then shard0's tokens (split per k-subtile to
            # spread over DMA queues), then everything else.  All input DMAs on
            # nc.sync (HWDGE): gpsimd SWDGE dispatch is ~2x slower.
            wg_tiles = {}
            wu_tiles = {}
            wg_tiles[0] = wpool.tile([P, DSUB, P], F16, tag="wg", name="wg0")
            nc.sync.dma_start(wg_tiles[0][:], wg_d[0])
            xt = []
            xt.append(const_pool.tile([P, DSUB, Cs[0]], F16, name="xt0"))
            for k in range(DSUB):
                nc.sync.dma_start(xt[0][:, k], xt_d[0][:, k])
            wu_tiles[0] = wpool.tile([P, DSUB, P], F16, tag="wu", name="wu0")
            nc.sync.dma_start(wu_tiles[0][:], wu_d[0])
            wt_sb = [const_pool.tile([P, Cs[0] // P], F32, name="wt0")]
            nc.sync.dma_start(wt_sb[0][:], wt_d[0][:])

            # later shards' token feeds + combine weights, loaded as reached
            for s in range(1, NSPLIT):
                xt.append(const_pool.tile([P, DSUB, Cs[s]], F16, name=f"xt{s}"))
                wt_sb.append(
                    const_pool.tile([P, Cs[s] // P], F32, name=f"wt{s}")
                )

            wd_sb = const_pool.tile([P, FBLK, NDT, NDN], F16)

            act = [
                act_pool.tile([P, FBH, Cs[s]], F16, name=f"act{s}")
                for s in range(NSPLIT)
            ]

            for s in range(NSPLIT):
                C = Cs[s]
                csizes = _chunks(C)
                if s >= 1:
                    for k in range(DSUB):
                        nc.sync.dma_start(xt[s][:, k], xt_d[s][:, k])
                    nc.sync.dma_start(wt_sb[s][:], wt_d[s][:])

                # Phase 1: gateT/upT per FF-block of this shard, fused silu*up
                for fbl in range(FBH):
                    fb = s * FBH + fbl
                    wg_t = wg_tiles.pop(fb)
                    wu_t = wu_tiles.pop(fb)
                    if fb + 1 < FBLK:
                        nwg = wpool.tile([P, DSUB, P], F16, tag="wg", name="wg")
                        nc.sync.dma_start(nwg[:], wg_d[fb + 1])
                        nwu = wpool.tile([P, DSUB, P], F16, tag="wu", name="wu")
                        nc.sync.dma_start(nwu[:], wu_d[fb + 1])
                        wg_tiles[fb + 1] = nwg
                        wu_tiles[fb + 1] = nwu
                    # stream one wd block per iteration
                    nc.sync.dma_start(wd_sb[:, fb], wd_d[fb])

                    col = 0
                    for cs in csizes:
                        pg = psum_pool.tile([P, NDN], F32, tag="pg", name="pg")[:, :cs]
                        pu = psum_pool.tile([P, NDN], F32, tag="pu", name="pu")[:, :cs]
                        for k in range(DSUB):
                            nc.tensor.matmul(
                                pg,
                                wg_t[:, k, :],
                                xt[s][:, k, col : col + cs],
                                start=(k == 0),
                                stop=(k == DSUB - 1),
                            )
                        for k in range(DSUB):
                            nc.tensor.matmul(
                                pu,
                                wu_t[:, k, :],
                                xt[s][:, k, col : col + cs],
                                start=(k == 0),
                                stop=(k == DSUB - 1),
                            )
                        sg = sg_pool.tile([P, NDN], F32, tag="sg", name="sg")[:, :cs]
                        nc.scalar.activation(
                            sg, pg, mybir.ActivationFunctionType.Silu
                        )
                        nc.vector.tensor_mul(act[s][:, fbl, col : col + cs], sg, pu)
                        col += cs

                # Phase 2: y[t, d] = (actT.T @ WdT_half) * combine_weight[t]
                for tb in range(C // P):
                    for dti in range(NDT):
                        py = psum_y_pool.tile([P, NDN], F32, tag="py")
                        for fs in range(FBH):
                            nc.tensor.matmul(
                                py,
                                act[s][:, fs, tb * P : (tb + 1) * P],
                                wd_sb[:, s * FBH + fs, dti, :],
                                start=(fs == 0),
                                stop=(fs == FBH - 1),
                            )
                        y_sb = y_pool.tile([P, NDN], F32, tag="ysb")
                        nc.vector.tensor_scalar_mul(
                            y_sb, py, wt_sb[s][:, tb : tb + 1]
                        )
                        nc.sync.dma_start(
                            y_d[s][tb, :, dti * NDN : (dti + 1) * NDN], y_sb
                        )

    nc.compile()
    return nc


def _shard_feed(h16, gp, up, dp, combine, routed, e, piece, C):
    """Build one (expert, FF-half) shard's DMA feeds, pre-laid-out to match the
    kernel's SBUF tile layouts exactly (every DMA contiguous)."""
    r = routed[e]
    n_e = len(r)
    idx_pad = np.zeros(C, np.int64)
    idx_pad[:n_e] = r
    wt_pad = np.zeros(C, np.float32)
    wt_pad[:n_e] = combine[e, r]
    hs = slice(piece * FH, (piece + 1) * FH)

    xg = h16[idx_pad]  # [C, D] fp16
    xt_feed = np.ascontiguousarray(xg.reshape(C, DSUB, P).transpose(2, 1, 0))
    wg_feed = np.ascontiguousarray(
        gp[e][hs, :].astype(np.float16).reshape(FBH, P, DSUB, P).transpose(0, 3, 2, 1)
    )
    wu_feed = np.ascontiguousarray(
        up[e][hs, :].astype(np.float16).reshape(FBH, P, DSUB, P).transpose(0, 3, 2, 1)
    )
    # wd_feed[fs, p, dt, dn] = down_proj[e][dt*NDN+dn, half*FH + fs*P+p]
    wd_feed = np.ascontiguousarray(
        dp[e][:, hs].astype(np.float16).reshape(NDT, NDN, FBH, P).transpose(2, 3, 0, 1)
    )
    wt_feed = np.ascontiguousarray(wt_pad.reshape(C // P, P).T)
    return xt_feed, wg_feed, wu_feed, wd_feed, wt_feed


def kernel(hidden_states, top_k_index, top_k_weights, gate_proj, up_proj, down_proj):
    h = np.ascontiguousarray(np.asarray(hidden_states, dtype=np.float32))
    idx = np.asarray(top_k_index)
    wts = np.asarray(top_k_weights, dtype=np.float32)
    gp = np.asarray(gate_proj, dtype=np.float32)
    up = np.asarray(up_proj, dtype=np.float32)
    dp = np.asarray(down_proj, dtype=np.float32)
    assert h.shape == (T, D) and idx.shape == (T, TOPK)
    assert gp.shape == (E, FF, D) and dp.shape == (E, D, FF)

    # combine[e, t] = sum_k wts[t, k] * (idx[t, k] == e)
    combine = np.zeros((E, T), np.float32)
    for k in range(TOPK):
        np.add.at(combine, (idx[:, k], np.arange(T)), wts[:, k])

    routed = [np.nonzero(combine[e] > 0)[0] for e in range(E)]
    cnt = [len(r) for r in routed]

    # E*NSPLIT (expert, FF-piece) shards, sorted by routed count; slot s takes
    # ranks [s*E, (s+1)*E) so every core pairs one shard from each size tier.
    shards = sorted(
        ((e, piece) for e in range(E) for piece in range(NSPLIT)),
        key=lambda sh: -cnt[sh[0]],
    )
    slots = [shards[s * E : (s + 1) * E] for s in range(NSPLIT)]
    pad = lambda n: max(P, -(-n // P) * P)
    Cs = tuple(pad(max(cnt[e] for e, _ in slot)) for slot in slots)

    h16 = h.astype(np.float16)
    in_maps = []
    for core in range(E):
        m = {}
        wg_parts, wu_parts, wd_parts = [], [], []
        for s in range(NSPLIT):
            e, piece = slots[s][core]
            xt_f, wg_f, wu_f, wd_f, wt_f = _shard_feed(
                h16, gp, up, dp, combine, routed, e, piece, Cs[s]
            )
            m[f"xt{s}"] = xt_f
            m[f"wt{s}"] = wt_f
            wg_parts.append(wg_f)
            wu_parts.append(wu_f)
            wd_parts.append(wd_f)
        m["wg"] = np.ascontiguousarray(np.concatenate(wg_parts, axis=0))
        m["wu"] = np.ascontiguousarray(np.concatenate(wu_parts, axis=0))
        m["wd"] = np.ascontiguousarray(np.concatenate(wd_parts, axis=0))
        in_maps.append(m)

    ys = _run_on_device(Cs, in_maps)

    out = np.zeros((T, D), np.float32)
    for core in range(E):
        for s in range(NSPLIT):
            e, piece = slots[s][core]
            r = routed[e]
            out[r] += ys[core][s].reshape(Cs[s], D)[: len(r)]
    return out


def _have_axon() -> bool:
    """The bass kernel executes via PJRT on the axon-tunneled NeuronCores.
    If the calling process pinned JAX_PLATFORMS=cpu (hiding them), fall back
    to a clean subprocess."""
    try:
        import jax

        return sum(1 for d in jax.devices() if getattr(d, "platform", "") != "cpu") >= E
    except Exception:
        return False


def _run_on_device(Cs: tuple, in_maps: list) -> list:
    global last_results
    if _have_axon():
        if Cs not in _program_cache:
            _program_cache[Cs] = _build_program(Cs)
        nc = _program_cache[Cs]
        last_results = run_bass_kernel_spmd(nc, in_maps, core_ids=list(range(E)))
        return [
            [last_results.results[c][f"y{s}"] for s in range(NSPLIT)]
            for c in range(E)
        ]

    import pickle
    import subprocess
    import tempfile

    d = tempfile.mkdtemp()
    inp, outp = os.path.join(d, "in.pkl"), os.path.join(d, "out.pkl")
    with open(inp, "wb") as f:
        pickle.dump((Cs, in_maps), f)
    env = dict(os.environ)
    env.pop("JAX_PLATFORMS", None)
    subprocess.run(
        [sys.executable, os.path.abspath(__file__), "--device-run", inp, outp],
        check=True,
        env=env,
    )
    with open(outp, "rb") as f:
        return pickle.load(f)


if __name__ == "__main__" and "--device-run" in sys.argv:
    import pickle

    _inp, _outp = sys.argv[2], sys.argv[3]
    with open(_inp, "rb") as f:
        _Cs, _in_maps = pickle.load(f)
    _nc = _build_program(_Cs)
    _res = run_bass_kernel_spmd(_nc, _in_maps, core_ids=list(range(E)))
    with open(_outp, "wb") as f:
        pickle.dump(
            [[_res.results[c][f"y{s}"] for s in range(NSPLIT)] for c in range(E)],
            f,
        )


# revision 28
# speedup vs baseline: 1.0437x; 1.0437x over previous
"""MoE experts kernel for Trainium2 (Bass/Tile), expert-parallel across 8 NeuronCores.

Problem: nn_CompressedMoeExperts — T=2048 tokens, D=1024, FF=1536, E=8 experts,
top-k=2.  out[t] = sum_e combine[e,t] * (silu(h[t] @ Wg[e].T) * (h[t] @ Wu[e].T)) @ Wd[e].T

Sharding: expert-parallel with FF-split load balancing.  Each expert's MLP is
split into two independent shards along the FF dimension (rows of Wg/Wu,
columns of Wd — their partial down-projection outputs simply add).  The 16
shards are sorted by routed-token count and dealt out so every core gets one
"big" and one "small" shard; per-core compute is then proportional to
C0+C1 (max big + max small capacity) instead of 2*C_max, which removes the
hot-expert straggler penalty under skewed routing.  Token dispatch (gather by
top_k_index) and the weighted combine scatter-add happen on the host as part
of sharding/unsharding; the combine weight itself is applied on-device.

Matmul operands are fp16 (halves HBM traffic vs fp32, 1 cycle/row on the PE,
fast weight loads — unlike float32r which forces a ~190ns LDWEIGHTS reload per
matmul), accumulating in fp32 PSUM.  Values are far inside fp16 range and the
10-bit mantissa keeps L2 relative error ~5e-4.  All DMA feeds are pre-laid-out
on the host into exact SBUF tile layouts so every DMA is contiguous, split into
~0.5MB pieces to spread across DMA queues.  A short run of dummy matmuls warms
the PE clock (HAM gate: 1.2 GHz until ~3.4us of sustained activity) while the
first DMAs stage.
"""

import os
import sys

sys.path.insert(0, "/opt/trn_rl_repo")

import numpy as np

import concourse.bass as bass
import concourse.mybir as mybir
import concourse.tile as tile
from concourse import bacc
from concourse.bass_utils import run_bass_kernel_spmd

# Fixed problem shape
T, D, FF, E, TOPK = 2048, 1024, 1536, 8, 2
P = 128
DSUB = D // P     # 8   k-subtiles over the D contraction
FBLK = FF // P    # 12  128-row blocks over the full FF dimension
# FF-shards per expert (= shard slots per core).  3 balances two effects:
# more shards -> tighter load balance (slot capacity = per-tier max routed
# count), fewer shards -> longer phase-2 PSUM accumulation groups so the
# per-group Vector-engine eviction (~0.75us) stays hidden behind the PE
# (NSPLIT=4 made phase 2 eviction-bound and measured ~14us slower).
NSPLIT = 3
FBH = FBLK // NSPLIT   # 128-row FF blocks per shard
FH = FF // NSPLIT      # FF columns per shard
NDN = 512         # free-dim tile for the down projection
NDT = D // NDN    # 2

F32 = mybir.dt.float32
F16 = mybir.dt.float16

_program_cache: dict[tuple, "bass.Bass"] = {}
last_results = None  # BassKernelResults of the most recent run (for profiling)


def _chunks(C: int) -> list[int]:
    """Split C (multiple of 128) into matmul moving-dim chunks of <=512
    (PSUM bank limit for fp32 accumulation)."""
    nb = C // P
    n = -(-nb * P // 512)  # ceil(C/512)
    base, rem = divmod(nb, n)
    return [(base + (1 if i < rem else 0)) * P for i in range(n)]


def _build_program(Cs: tuple) -> "bass.Bass":
    nc = bacc.Bacc(None, target_bir_lowering=False)

    xt_d = [
        nc.dram_tensor(f"xt{s}", [P, DSUB, Cs[s]], F16, kind="ExternalInput")
        for s in range(NSPLIT)
    ]
    wg_d = nc.dram_tensor("wg", [FBLK, P, DSUB, P], F16, kind="ExternalInput")
    wu_d = nc.dram_tensor("wu", [FBLK, P, DSUB, P], F16, kind="ExternalInput")
    wd_d = nc.dram_tensor("wd", [FBLK, P, NDT, NDN], F16, kind="ExternalInput")
    wt_d = [
        nc.dram_tensor(f"wt{s}", [P, Cs[s] // P], F32, kind="ExternalInput")
        for s in range(NSPLIT)
    ]
    y_d = [
        nc.dram_tensor(f"y{s}", [Cs[s] // P, P, D], F32, kind="ExternalOutput")
        for s in range(NSPLIT)
    ]

    with tile.TileContext(nc) as tc:
        with (
            tc.tile_pool(name="const", bufs=1) as const_pool,
            tc.tile_pool(name="wpool", bufs=3) as wpool,
            tc.tile_pool(name="actp", bufs=1) as act_pool,
            tc.tile_pool(name="sgp", bufs=3) as sg_pool,
            tc.tile_pool(name="yp", bufs=3) as y_pool,
            tc.tile_pool(name="psum", bufs=2, space="PSUM") as psum_pool,
            tc.tile_pool(name="psum_y", bufs=3, space="PSUM") as psum_y_pool,
            tc.tile_pool(name="psum_w", bufs=1, space="PSUM") as psum_w_pool,
        ):
            # HAM pre-warm: dummy matmuls (only dep: the memset) keep the PE
            # busy while the first DMAs stage, so real matmuls start at 2.4GHz.
            warm_in = const_pool.tile([P, NDN], F16)
            nc.vector.memset(warm_in[:], 0.0)
            warm_ps = psum_w_pool.tile([P, NDN], F32)
            for _ in range(10):
                nc.tensor.matmul(warm_ps[:], warm_in[:, :P], warm_in[:])

            # First weight block, then shard0's tokens (split per k-subtile to
            # spread over DMA queues), then everything else.  All input DMAs on
            # nc.sync (HWDGE): gpsimd SWDGE dispatch is ~2x slower.
            wg_tiles = {}
            wu_tiles = {}
            wg_tiles[0] = wpool.tile([P, DSUB, P], F16, tag="wg", name="wg0")
            nc.sync.dma_start(wg_tiles[0][:], wg_d[0])
            xt = []
            xt.append(const_pool.tile([P, DSUB, Cs[0]], F16, name="xt0"))
            for k in range(DSUB):
                nc.sync.dma_start(xt[0][:, k], xt_d[0][:, k])
            wu_tiles[0] = wpool.tile([P, DSUB, P], F16, tag="wu", name="wu0")
            nc.sync.dma_start(wu_tiles[0][:], wu_d[0])
            wt_sb = [const_pool.tile([P, Cs[0] // P], F32, name="wt0")]
            nc.sync.dma_start(wt_sb[0][:], wt_d[0][:])

            # later shards' token feeds + combine weights, loaded as reached
            for s in range(1, NSPLIT):
                xt.append(const_pool.tile([P, DSUB, Cs[s]], F16, name=f"xt{s}"))
                wt_sb.append(
                    const_pool.tile([P, Cs[s] // P], F32, name=f"wt{s}")
                )

            wd_sb = const_pool.tile([P, FBLK, NDT, NDN], F16)

            act = [
                act_pool.tile([P, FBH, Cs[s]], F16, name=f"act{s}")
                for s in range(NSPLIT)
            ]

            for s in range(NSPLIT):
                C = Cs[s]
                csizes = _chunks(C)
                if s >= 1:
                    for k in range(DSUB):
                        nc.sync.dma_start(xt[s][:, k], xt_d[s][:, k])
                    nc.sync.dma_start(wt_sb[s][:], wt_d[s][:])

                # Phase 1: gateT/upT per FF-block of this shard, fused silu*up
                for fbl in range(FBH):
                    fb = s * FBH + fbl
                    wg_t = wg_tiles.pop(fb)
                    wu_t = wu_tiles.pop(fb)
                    if fb + 1 < FBLK:
                        nwg = wpool.tile([P, DSUB, P], F16, tag="wg", name="wg")
                        nc.sync.dma_start(nwg[:], wg_d[fb + 1])
                        nwu = wpool.tile([P, DSUB, P], F16, tag="wu", name="wu")
                        nc.sync.dma_start(nwu[:], wu_d[fb + 1])
                        wg_tiles[fb + 1] = nwg
                        wu_tiles[fb + 1] = nwu
                    # stream one wd block per iteration
                    nc.sync.dma_start(wd_sb[:, fb], wd_d[fb])

                    col = 0
                    for cs in csizes:
                        pg = psum_pool.tile([P, NDN], F32, tag="pg", name="pg")[:, :cs]
                        pu = psum_pool.tile([P, NDN], F32, tag="pu", name="pu")[:, :cs]
                        for k in range(DSUB):
                            nc.tensor.matmul(
                                pg,
                                wg_t[:, k, :],
                                xt[s][:, k, col : col + cs],
                                start=(k == 0),
                                stop=(k == DSUB - 1),
                            )
                        for k in range(DSUB):
                            nc.tensor.matmul(
                                pu,
                                wu_t[:, k, :],
                                xt[s][:, k, col : col + cs],
                                start=(k == 0),
                                stop=(k == DSUB - 1),
                            )
                        sg = sg_pool.tile([P, NDN], F32, tag="sg", name="sg")[:, :cs]
                        nc.scalar.activation(
                            sg, pg, mybir.ActivationFunctionType.Silu
                        )
                        nc.vector.tensor_mul(act[s][:, fbl, col : col + cs], sg, pu)
                        col += cs

                # Phase 2: y[t, d] = (actT.T @ WdT_half) * combine_weight[t]
                for tb in range(C // P):
                    for dti in range(NDT):
                        py = psum_y_pool.tile([P, NDN], F32, tag="py")
                        for fs in range(FBH):
                            nc.tensor.matmul(
                                py,
                                act[s][:, fs, tb * P : (tb + 1) * P],
                                wd_sb[:, s * FBH + fs, dti, :],
                                start=(fs == 0),
                                stop=(fs == FBH - 1),
                            )
                        y_sb = y_pool.tile([P, NDN], F32, tag="ysb")
                        nc.vector.tensor_scalar_mul(
                            y_sb, py, wt_sb[s][:, tb : tb + 1]
                        )
                        nc.sync.dma_start(
                            y_d[s][tb, :, dti * NDN : (dti + 1) * NDN], y_sb
                        )

    nc.compile()
    return nc


def _shard_feed(h16, gp, up, dp, combine, routed, e, piece, C):
    """Build one (expert, FF-half) shard's DMA feeds, pre-laid-out to match the
    kernel's SBUF tile layouts exactly (every DMA contiguous)."""
    r = routed[e]
    n_e = len(r)
    idx_pad = np.zeros(C, np.int64)
    idx_pad[:n_e] = r
    wt_pad = np.zeros(C, np.float32)
    wt_pad[:n_e] = combine[e, r]
    hs = slice(piece * FH, (piece + 1) * FH)

    xg = h16[idx_pad]  # [C, D] fp16
    xt_feed = np.ascontiguousarray(xg.reshape(C, DSUB, P).transpose(2, 1, 0))
    wg_feed = np.ascontiguousarray(
        gp[e][hs, :].astype(np.float16).reshape(FBH, P, DSUB, P).transpose(0, 3, 2, 1)
    )
    wu_feed = np.ascontiguousarray(
        up[e][hs, :].astype(np.float16).reshape(FBH, P, DSUB, P).transpose(0, 3, 2, 1)
    )
    # wd_feed[fs, p, dt, dn] = down_proj[e][dt*NDN+dn, half*FH + fs*P+p]
    wd_feed = np.ascontiguousarray(
        dp[e][:, hs].astype(np.float16).reshape(NDT, NDN, FBH, P).transpose(2, 3, 0, 1)
    )
    wt_feed = np.ascontiguousarray(wt_pad.reshape(C // P, P).T)
    return xt_feed, wg_feed, wu_feed, wd_feed, wt_feed


def kernel(hidden_states, top_k_index, top_k_weights, gate_proj, up_proj, down_proj):
    h = np.ascontiguousarray(np.asarray(hidden_states, dtype=np.float32))
    idx = np.asarray(top_k_index)
    wts = np.asarray(top_k_weights, dtype=np.float32)
    gp = np.asarray(gate_proj, dtype=np.float32)
    up = np.asarray(up_proj, dtype=np.float32)
    dp = np.asarray(down_proj, dtype=np.float32)
    assert h.shape == (T, D) and idx.shape == (T, TOPK)
    assert gp.shape == (E, FF, D) and dp.shape == (E, D, FF)

    # combine[e, t] = sum_k wts[t, k] * (idx[t, k] == e)
    combine = np.zeros((E, T), np.float32)
    for k in range(TOPK):
        np.add.at(combine, (idx[:, k], np.arange(T)), wts[:, k])

    routed = [np.nonzero(combine[e] > 0)[0] for e in range(E)]
    cnt = [len(r) for r in routed]

    # E*NSPLIT (expert, FF-piece) shards, sorted by routed count; slot s takes
    # ranks [s*E, (s+1)*E) so every core pairs one shard from each size tier.
    shards = sorted(
        ((e, piece) for e in range(E) for piece in range(NSPLIT)),
        key=lambda sh: -cnt[sh[0]],
    )
    slots = [shards[s * E : (s + 1) * E] for s in range(NSPLIT)]
    pad = lambda n: max(P, -(-n // P) * P)
    Cs = tuple(pad(max(cnt[e] for e, _ in slot)) for slot in slots)

    h16 = h.astype(np.float16)
    in_maps = []
    for core in range(E):
        m = {}
        wg_parts, wu_parts, wd_parts = [], [], []
        for s in range(NSPLIT):
            e, piece = slots[s][core]
            xt_f, wg_f, wu_f, wd_f, wt_f = _shard_feed(
                h16, gp, up, dp, combine, routed, e, piece, Cs[s]
            )
            m[f"xt{s}"] = xt_f
            m[f"wt{s}"] = wt_f
            wg_parts.append(wg_f)
            wu_parts.append(wu_f)
            wd_parts.append(wd_f)
        m["wg"] = np.ascontiguousarray(np.concatenate(wg_parts, axis=0))
        m["wu"] = np.ascontiguousarray(np.concatenate(wu_parts, axis=0))
        m["wd"] = np.ascontiguousarray(np.concatenate(wd_parts, axis=0))
        in_maps.append(m)

    ys = _run_on_device(Cs, in_maps)

    out = np.zeros((T, D), np.float32)
    for core in range(E):
        for s in range(NSPLIT):
            e, piece = slots[s][core]
            r = routed[e]
            out[r] += ys[core][s].reshape(Cs[s], D)[: len(r)]
    return out


def _have_axon() -> bool:
    """The bass kernel executes via PJRT on the axon-tunneled NeuronCores.
    If the calling process pinned JAX_PLATFORMS=cpu (hiding them), fall back
    to a clean subprocess."""
    try:
        import jax

        return sum(1 for d in jax.devices() if getattr(d, "platform", "") != "cpu") >= E
    except Exception:
        return False


def _run_on_device(Cs: tuple, in_maps: list) -> list:
    global last_results
    if _have_axon():
        if Cs not in _program_cache:
            _program_cache[Cs] = _build_program(Cs)
        nc = _program_cache[Cs]
        last_results = run_bass_kernel_spmd(nc, in_maps, core_ids=list(range(E)))
        return [
            [last_results.results[c][f"y{s}"] for s in range(NSPLIT)]
            for c in range(E)
        ]

    import pickle
    import subprocess
    import tempfile

    d = tempfile.mkdtemp()
    inp, outp = os.path.join(d, "in.pkl"), os.path.join(d, "out.pkl")
    with open(inp, "wb") as f:
        pickle.dump((Cs, in_maps), f)
    env = dict(os.environ)
    env.pop("JAX_PLATFORMS", None)
    subprocess.run(
        [sys.executable, os.path.abspath(__file__), "--device-run", inp, outp],
        check=True,
        env=env,
    )
    with open(outp, "rb") as f:
        return pickle.load(f)


if __name__ == "__main__" and "--device-run" in sys.argv:
    import pickle

    _inp, _outp = sys.argv[2], sys.argv[3]
    with open(_inp, "rb") as f:
        _Cs, _in_maps = pickle.load(f)
    _nc = _build_program(_Cs)
    _res = run_bass_kernel_spmd(_nc, _in_maps, core_ids=list(range(E)))
    with open(_outp, "wb") as f:
        pickle.dump(
            [[_res.results[c][f"y{s}"] for s in range(NSPLIT)] for c in range(E)],
            f,
        )


# revision 35
# speedup vs baseline: 1.1508x; 1.1026x over previous
"""MoE experts kernel for Trainium2 (Bass/Tile), expert-parallel across 8 NeuronCores.

Problem: nn_CompressedMoeExperts — T=2048 tokens, D=1024, FF=1536, E=8 experts,
top-k=2.  out[t] = sum_e combine[e,t] * (silu(h[t] @ Wg[e].T) * (h[t] @ Wu[e].T)) @ Wd[e].T

Sharding: expert-parallel with FF-split load balancing.  Each expert's MLP is
split into two independent shards along the FF dimension (rows of Wg/Wu,
columns of Wd — their partial down-projection outputs simply add).  The 16
shards are sorted by routed-token count and dealt out so every core gets one
"big" and one "small" shard; per-core compute is then proportional to
C0+C1 (max big + max small capacity) instead of 2*C_max, which removes the
hot-expert straggler penalty under skewed routing.  Token dispatch (gather by
top_k_index) and the weighted combine scatter-add happen on the host as part
of sharding/unsharding; the combine weight itself is applied on-device.

Matmul operands are fp16 (halves HBM traffic vs fp32, 1 cycle/row on the PE,
fast weight loads — unlike float32r which forces a ~190ns LDWEIGHTS reload per
matmul), accumulating in fp32 PSUM.  Values are far inside fp16 range and the
10-bit mantissa keeps L2 relative error ~5e-4.  All DMA feeds are pre-laid-out
on the host into exact SBUF tile layouts so every DMA is contiguous, split into
~0.5MB pieces to spread across DMA queues.  A short run of dummy matmuls warms
the PE clock (HAM gate: 1.2 GHz until ~3.4us of sustained activity) while the
first DMAs stage.
"""

import os
import sys

sys.path.insert(0, "/opt/trn_rl_repo")

import numpy as np

import concourse.bass as bass
import concourse.mybir as mybir
import concourse.tile as tile
from concourse import bacc
from concourse.bass_utils import run_bass_kernel_spmd

# Fixed problem shape
T, D, FF, E, TOPK = 2048, 1024, 1536, 8, 2
P = 128
DSUB = D // P     # 8   k-subtiles over the D contraction
FBLK = FF // P    # 12  128-row blocks over the full FF dimension
# FF-shards per expert (= shard slots per core).  2 balances two effects:
# more shards -> tighter load balance (slot capacity = per-tier max routed
# count), fewer shards -> longer phase-2 PSUM accumulation groups so the
# per-group Vector-engine eviction (~0.75us) stays hidden behind the PE.
# Measured: NSPLIT=2 90.8us, NSPLIT=3 100.4us, NSPLIT=4 104.7us — the thinner
# phase-2 groups of 3/4-way splits made eviction the critical path.
NSPLIT = 2
FBH = FBLK // NSPLIT   # 128-row FF blocks per shard
FH = FF // NSPLIT      # FF columns per shard
NDN = 512         # free-dim tile for the down projection
NDT = D // NDN    # 2

F32 = mybir.dt.float32
F16 = mybir.dt.float16

_program_cache: dict[tuple, "bass.Bass"] = {}
last_results = None  # BassKernelResults of the most recent run (for profiling)


def _chunks(C: int) -> list[int]:
    """Split C (multiple of 128) into matmul moving-dim chunks of <=512
    (PSUM bank limit for fp32 accumulation)."""
    nb = C // P
    n = -(-nb * P // 512)  # ceil(C/512)
    base, rem = divmod(nb, n)
    return [(base + (1 if i < rem else 0)) * P for i in range(n)]


def _build_program(Cs: tuple) -> "bass.Bass":
    nc = bacc.Bacc(None, target_bir_lowering=False)

    xt_d = [
        nc.dram_tensor(f"xt{s}", [P, DSUB, Cs[s]], F16, kind="ExternalInput")
        for s in range(NSPLIT)
    ]
    wg_d = nc.dram_tensor("wg", [FBLK, P, DSUB, P], F16, kind="ExternalInput")
    wu_d = nc.dram_tensor("wu", [FBLK, P, DSUB, P], F16, kind="ExternalInput")
    wd_d = nc.dram_tensor("wd", [FBLK, P, NDT, NDN], F16, kind="ExternalInput")
    wt_d = [
        nc.dram_tensor(f"wt{s}", [P, Cs[s] // P], F32, kind="ExternalInput")
        for s in range(NSPLIT)
    ]
    y_d = [
        nc.dram_tensor(f"y{s}", [Cs[s] // P, P, D], F32, kind="ExternalOutput")
        for s in range(NSPLIT)
    ]

    with tile.TileContext(nc) as tc:
        with (
            tc.tile_pool(name="const", bufs=1) as const_pool,
            tc.tile_pool(name="wpool", bufs=3) as wpool,
            tc.tile_pool(name="actp", bufs=1) as act_pool,
            tc.tile_pool(name="sgp", bufs=3) as sg_pool,
            tc.tile_pool(name="yp", bufs=3) as y_pool,
            tc.tile_pool(name="psum", bufs=2, space="PSUM") as psum_pool,
            tc.tile_pool(name="psum_y", bufs=3, space="PSUM") as psum_y_pool,
            tc.tile_pool(name="psum_w", bufs=1, space="PSUM") as psum_w_pool,
        ):
            # HAM pre-warm: dummy matmuls (only dep: the memset) keep the PE
            # busy while the first DMAs stage, so real matmuls start at 2.4GHz.
            warm_in = const_pool.tile([P, NDN], F16)
            nc.vector.memset(warm_in[:], 0.0)
            warm_ps = psum_w_pool.tile([P, NDN], F32)
            for _ in range(10):
                nc.tensor.matmul(warm_ps[:], warm_in[:, :P], warm_in[:])

            # First weight block, then shard0's tokens (split per k-subtile to
            # spread over DMA queues), then everything else.  All input DMAs on
            # nc.sync (HWDGE): gpsimd SWDGE dispatch is ~2x slower.
            wg_tiles = {}
            wu_tiles = {}
            wg_tiles[0] = wpool.tile([P, DSUB, P], F16, tag="wg", name="wg0")
            nc.sync.dma_start(wg_tiles[0][:], wg_d[0])
            xt = []
            xt.append(const_pool.tile([P, DSUB, Cs[0]], F16, name="xt0"))
            for k in range(DSUB):
                nc.sync.dma_start(xt[0][:, k], xt_d[0][:, k])
            wu_tiles[0] = wpool.tile([P, DSUB, P], F16, tag="wu", name="wu0")
            nc.sync.dma_start(wu_tiles[0][:], wu_d[0])
            # fb=1 weights go out with the startup batch too: dispatched after
            # the fb-loop's other upfront DMAs they arrive ~1us after the PE
            # finishes fb=0 (measured 1.08us stall).
            wg_tiles[1] = wpool.tile([P, DSUB, P], F16, tag="wg", name="wg1")
            nc.sync.dma_start(wg_tiles[1][:], wg_d[1])
            wu_tiles[1] = wpool.tile([P, DSUB, P], F16, tag="wu", name="wu1")
            nc.sync.dma_start(wu_tiles[1][:], wu_d[1])
            wt_sb = [const_pool.tile([P, Cs[0] // P], F32, name="wt0")]
            nc.sync.dma_start(wt_sb[0][:], wt_d[0][:])

            # later shards' token feeds + combine weights, loaded as reached
            for s in range(1, NSPLIT):
                xt.append(const_pool.tile([P, DSUB, Cs[s]], F16, name=f"xt{s}"))
                wt_sb.append(
                    const_pool.tile([P, Cs[s] // P], F32, name=f"wt{s}")
                )

            wd_sb = const_pool.tile([P, FBLK, NDT, NDN], F16)

            act = [
                act_pool.tile([P, FBH, Cs[s]], F16, name=f"act{s}")
                for s in range(NSPLIT)
            ]

            for s in range(NSPLIT):
                C = Cs[s]
                csizes = _chunks(C)
                if s >= 1:
                    for k in range(DSUB):
                        nc.sync.dma_start(xt[s][:, k], xt_d[s][:, k])
                    nc.sync.dma_start(wt_sb[s][:], wt_d[s][:])

                # Phase 1: gateT/upT per FF-block of this shard, fused silu*up
                for fbl in range(FBH):
                    fb = s * FBH + fbl
                    wg_t = wg_tiles.pop(fb)
                    wu_t = wu_tiles.pop(fb)
                    if fb + 2 < FBLK:
                        nwg = wpool.tile([P, DSUB, P], F16, tag="wg", name="wg")
                        nc.sync.dma_start(nwg[:], wg_d[fb + 2])
                        nwu = wpool.tile([P, DSUB, P], F16, tag="wu", name="wu")
                        nc.sync.dma_start(nwu[:], wu_d[fb + 2])
                        wg_tiles[fb + 2] = nwg
                        wu_tiles[fb + 2] = nwu
                    # stream one wd block per iteration
                    nc.sync.dma_start(wd_sb[:, fb], wd_d[fb])

                    col = 0
                    for cs in csizes:
                        pg = psum_pool.tile([P, NDN], F32, tag="pg", name="pg")[:, :cs]
                        pu = psum_pool.tile([P, NDN], F32, tag="pu", name="pu")[:, :cs]
                        for k in range(DSUB):
                            nc.tensor.matmul(
                                pg,
                                wg_t[:, k, :],
                                xt[s][:, k, col : col + cs],
                                start=(k == 0),
                                stop=(k == DSUB - 1),
                            )
                        for k in range(DSUB):
                            nc.tensor.matmul(
                                pu,
                                wu_t[:, k, :],
                                xt[s][:, k, col : col + cs],
                                start=(k == 0),
                                stop=(k == DSUB - 1),
                            )
                        sg = sg_pool.tile([P, NDN], F32, tag="sg", name="sg")[:, :cs]
                        nc.scalar.activation(
                            sg, pg, mybir.ActivationFunctionType.Silu
                        )
                        nc.vector.tensor_mul(act[s][:, fbl, col : col + cs], sg, pu)
                        col += cs

                # Phase 2: y[t, d] = (actT.T @ WdT_half) * combine_weight[t]
                for tb in range(C // P):
                    for dti in range(NDT):
                        py = psum_y_pool.tile([P, NDN], F32, tag="py")
                        for fs in range(FBH):
                            nc.tensor.matmul(
                                py,
                                act[s][:, fs, tb * P : (tb + 1) * P],
                                wd_sb[:, s * FBH + fs, dti, :],
                                start=(fs == 0),
                                stop=(fs == FBH - 1),
                            )
                        y_sb = y_pool.tile([P, NDN], F32, tag="ysb")
                        nc.vector.tensor_scalar_mul(
                            y_sb, py, wt_sb[s][:, tb : tb + 1]
                        )
                        nc.sync.dma_start(
                            y_d[s][tb, :, dti * NDN : (dti + 1) * NDN], y_sb
                        )

    nc.compile()
    return nc


def _shard_feed(h16, gp, up, dp, combine, routed, e, piece, C):
    """Build one (expert, FF-half) shard's DMA feeds, pre-laid-out to match the
    kernel's SBUF tile layouts exactly (every DMA contiguous)."""
    r = routed[e]
    n_e = len(r)
    idx_pad = np.zeros(C, np.int64)
    idx_pad[:n_e] = r
    wt_pad = np.zeros(C, np.float32)
    wt_pad[:n_e] = combine[e, r]
    hs = slice(piece * FH, (piece + 1) * FH)

    xg = h16[idx_pad]  # [C, D] fp16
    xt_feed = np.ascontiguousarray(xg.reshape(C, DSUB, P).transpose(2, 1, 0))
    wg_feed = np.ascontiguousarray(
        gp[e][hs, :].astype(np.float16).reshape(FBH, P, DSUB, P).transpose(0, 3, 2, 1)
    )
    wu_feed = np.ascontiguousarray(
        up[e][hs, :].astype(np.float16).reshape(FBH, P, DSUB, P).transpose(0, 3, 2, 1)
    )
    # wd_feed[fs, p, dt, dn] = down_proj[e][dt*NDN+dn, half*FH + fs*P+p]
    wd_feed = np.ascontiguousarray(
        dp[e][:, hs].astype(np.float16).reshape(NDT, NDN, FBH, P).transpose(2, 3, 0, 1)
    )
    wt_feed = np.ascontiguousarray(wt_pad.reshape(C // P, P).T)
    return xt_feed, wg_feed, wu_feed, wd_feed, wt_feed


def kernel(hidden_states, top_k_index, top_k_weights, gate_proj, up_proj, down_proj):
    h = np.ascontiguousarray(np.asarray(hidden_states, dtype=np.float32))
    idx = np.asarray(top_k_index)
    wts = np.asarray(top_k_weights, dtype=np.float32)
    gp = np.asarray(gate_proj, dtype=np.float32)
    up = np.asarray(up_proj, dtype=np.float32)
    dp = np.asarray(down_proj, dtype=np.float32)
    assert h.shape == (T, D) and idx.shape == (T, TOPK)
    assert gp.shape == (E, FF, D) and dp.shape == (E, D, FF)

    # combine[e, t] = sum_k wts[t, k] * (idx[t, k] == e)
    combine = np.zeros((E, T), np.float32)
    for k in range(TOPK):
        np.add.at(combine, (idx[:, k], np.arange(T)), wts[:, k])

    routed = [np.nonzero(combine[e] > 0)[0] for e in range(E)]
    cnt = [len(r) for r in routed]

    # E*NSPLIT (expert, FF-piece) shards, sorted by routed count; slot s takes
    # ranks [s*E, (s+1)*E) so every core pairs one shard from each size tier.
    shards = sorted(
        ((e, piece) for e in range(E) for piece in range(NSPLIT)),
        key=lambda sh: -cnt[sh[0]],
    )
    slots = [shards[s * E : (s + 1) * E] for s in range(NSPLIT)]
    pad = lambda n: max(P, -(-n // P) * P)
    Cs = tuple(pad(max(cnt[e] for e, _ in slot)) for slot in slots)

    h16 = h.astype(np.float16)
    in_maps = []
    for core in range(E):
        m = {}
        wg_parts, wu_parts, wd_parts = [], [], []
        for s in range(NSPLIT):
            e, piece = slots[s][core]
            xt_f, wg_f, wu_f, wd_f, wt_f = _shard_feed(
                h16, gp, up, dp, combine, routed, e, piece, Cs[s]
            )
            m[f"xt{s}"] = xt_f
            m[f"wt{s}"] = wt_f
            wg_parts.append(wg_f)
            wu_parts.append(wu_f)
            wd_parts.append(wd_f)
        m["wg"] = np.ascontiguousarray(np.concatenate(wg_parts, axis=0))
        m["wu"] = np.ascontiguousarray(np.concatenate(wu_parts, axis=0))
        m["wd"] = np.ascontiguousarray(np.concatenate(wd_parts, axis=0))
        in_maps.append(m)

    ys = _run_on_device(Cs, in_maps)

    out = np.zeros((T, D), np.float32)
    for core in range(E):
        for s in range(NSPLIT):
            e, piece = slots[s][core]
            r = routed[e]
            out[r] += ys[core][s].reshape(Cs[s], D)[: len(r)]
    return out


def _have_axon() -> bool:
    """The bass kernel executes via PJRT on the axon-tunneled NeuronCores.
    If the calling process pinned JAX_PLATFORMS=cpu (hiding them), fall back
    to a clean subprocess."""
    try:
        import jax

        return sum(1 for d in jax.devices() if getattr(d, "platform", "") != "cpu") >= E
    except Exception:
        return False


def _run_on_device(Cs: tuple, in_maps: list) -> list:
    global last_results
    if _have_axon():
        if Cs not in _program_cache:
            _program_cache[Cs] = _build_program(Cs)
        nc = _program_cache[Cs]
        last_results = run_bass_kernel_spmd(nc, in_maps, core_ids=list(range(E)))
        return [
            [last_results.results[c][f"y{s}"] for s in range(NSPLIT)]
            for c in range(E)
        ]

    import pickle
    import subprocess
    import tempfile

    d = tempfile.mkdtemp()
    inp, outp = os.path.join(d, "in.pkl"), os.path.join(d, "out.pkl")
    with open(inp, "wb") as f:
        pickle.dump((Cs, in_maps), f)
    env = dict(os.environ)
    env.pop("JAX_PLATFORMS", None)
    subprocess.run(
        [sys.executable, os.path.abspath(__file__), "--device-run", inp, outp],
        check=True,
        env=env,
    )
    with open(outp, "rb") as f:
        return pickle.load(f)


if __name__ == "__main__" and "--device-run" in sys.argv:
    import pickle

    _inp, _outp = sys.argv[2], sys.argv[3]
    with open(_inp, "rb") as f:
        _Cs, _in_maps = pickle.load(f)
    _nc = _build_program(_Cs)
    _res = run_bass_kernel_spmd(_nc, _in_maps, core_ids=list(range(E)))
    with open(_outp, "wb") as f:
        pickle.dump(
            [[_res.results[c][f"y{s}"] for s in range(NSPLIT)] for c in range(E)],
            f,
        )
